# revision 1
# baseline (speedup 1.0000x reference)
"""PSRoIPool (position-sensitive ROI pooling) for Trainium2, 8 NeuronCores.

Problem (hardcoded):
  features [4, 392, 128, 128] f32, rois [512, 5] f32 (batch, x1, y1, x2, y2)
  out [512, 8, 7, 7] f32;  C = C_out(8) * 7 * 7;  spatial_scale = 1/16.

Sharding: by output channel c_out (8 cores). Core k owns feature channels
[49k, 49k+49) of every image (1/8 of the features, read exactly once) and
computes out[:, k, :, :] for ALL 512 rois.

Algorithm (per core), no gathers / no summed-area tables:
  bin_sum[r, c_s] = sum_h sum_w Mh[h; r, ph] * F[b(r), c_s, h, w] * Mw[w; r, pw]
  with c_s = ph*7 + pw (position-sensitive: one bin per slab channel).
  - stage 1 (PE): U[r, w] = sum_h Mh[h, r] * F[h, w]  (H-range mask as the
    stationary operand; 128 same-batch rois in the m dim)
  - stage 2 (DVE): fused multiply+reduce over w with the W-range mask
    (tensor_tensor_reduce), accumulated into a [128, 49] bins tile.
  - divide by bin pixel count, zero empty bins, DMA out.

Host side: group rois by batch index (stable sort) and pad each batch group
to a multiple of 128, so every 128-roi window is single-batch (PSUM base
partition must be 0). Per-roi bin bounds (hstart/hend/wstart/wend) are
precomputed in float32 math that bit-exactly mirrors the reference
(np.round == jnp.round, half-even). Output rows are un-permuted on the host.
"""

import numpy as np
from contextlib import ExitStack

N_IMG, C_FULL, H, W = 4, 392, 128, 128
R = 512
P = 7  # OUT_SIZE == GROUP
C_OUT = 8
C_SLAB = P * P  # 49 channels per core
SCALE = np.float32(0.0625)
NCORES = 8
RW = 128  # rois per window


# ---------------------------------------------------------------- host math
def _bounds(rois_padded: np.ndarray):
    """Bit-exact f32 mirror of the reference coordinate transform."""
    r = rois_padded.astype(np.float32)
    one = np.float32(1.0)
    rsw = np.round(r[:, 1]) * SCALE
    rsh = np.round(r[:, 2]) * SCALE
    rew = (np.round(r[:, 3]) + one) * SCALE
    reh = (np.round(r[:, 4]) + one) * SCALE
    roi_w = np.maximum(rew - rsw, np.float32(0.1))
    roi_h = np.maximum(reh - rsh, np.float32(0.1))
    bin_w = (roi_w / np.float32(P)).astype(np.float32)
    bin_h = (roi_h / np.float32(P)).astype(np.float32)
    p = np.arange(P, dtype=np.float32)
    hs = np.clip(np.floor(p[None, :] * bin_h[:, None] + rsh[:, None]), 0, H)
    he = np.clip(np.ceil((p[None, :] + one) * bin_h[:, None] + rsh[:, None]), 0, H)
    ws = np.clip(np.floor(p[None, :] * bin_w[:, None] + rsw[:, None]), 0, W)
    we = np.clip(np.ceil((p[None, :] + one) * bin_w[:, None] + rsw[:, None]), 0, W)
    return (hs.astype(np.float32), he.astype(np.float32),
            ws.astype(np.float32), we.astype(np.float32))


def _plan(batch: np.ndarray):
    """Group rois by batch, pad groups to multiples of RW.

    Returns (padmap, win_batch):
      padmap [R_PAD] int: original roi index backing each padded row
      win_batch [NWIN] int: the batch index of each 128-roi window
    """
    padmap = []
    win_batch = []
    for b in range(N_IMG):
        idx = np.nonzero(batch == b)[0]
        if len(idx) == 0:
            continue
        n_win = -(-len(idx) // RW)
        pad = n_win * RW - len(idx)
        rows = np.concatenate([idx, np.repeat(idx[:1], pad)])
        padmap.append(rows)
        win_batch.extend([b] * n_win)
    return np.concatenate(padmap), win_batch


# ---------------------------------------------------------------- device IR
def build_program(win_batch, repeat=1):
    import concourse.bass as bass
    import concourse.tile as tile
    from concourse import bacc, mybir

    f32 = mybir.dt.float32
    i32 = mybir.dt.int32
    Alu = mybir.AluOpType

    nwin = len(win_batch)
    r_pad = nwin * RW

    nc = bacc.Bacc("TRN2", target_bir_lowering=False, debug=False,
                   num_devices=NCORES)

    fslab = nc.dram_tensor("fslab", [N_IMG, C_SLAB, H, W], f32,
                           kind="ExternalInput").ap()
    # hbt[0] = hstart, hbt[1] = hend; ph-major flat layout [ph*r_pad + r]
    hbt = nc.dram_tensor("hbt", [2, P * r_pad], f32, kind="ExternalInput").ap()
    # r-major bounds: cols 0..6 = start per pw, cols 7..13 = end
    wb = nc.dram_tensor("wb", [r_pad, 2 * P], f32, kind="ExternalInput").ap()
    hb = nc.dram_tensor("hb", [r_pad, 2 * P], f32, kind="ExternalInput").ap()
    out = nc.dram_tensor("out", [r_pad, C_SLAB], f32, kind="ExternalOutput").ap()

    with tile.TileContext(nc) as tc, ExitStack() as ctx:
        consts = ctx.enter_context(tc.tile_pool(name="consts", bufs=1))
        fpool = ctx.enter_context(tc.tile_pool(name="fs", bufs=1))
        mpool = ctx.enter_context(tc.tile_pool(name="masks", bufs=2))
        spool = ctx.enter_context(tc.tile_pool(name="scratch", bufs=3))
        bpool = ctx.enter_context(tc.tile_pool(name="bins", bufs=2))
        opool = ctx.enter_context(tc.tile_pool(name="outw", bufs=2))
        psum = ctx.enter_context(tc.tile_pool(name="ps", bufs=3, space="PSUM"))

        # --- constants: partition iota (h index) and free-dim iota (w index)
        iota_p_i = consts.tile([128, 1], i32)
        nc.gpsimd.iota(iota_p_i[:], pattern=[[0, 1]], base=0, channel_multiplier=1)
        iota_p = consts.tile([128, 1], f32)
        nc.vector.tensor_copy(iota_p[:], iota_p_i[:])

        iota_f_i = consts.tile([128, P * W], i32)  # 0..127 repeated 7x, all parts
        nc.gpsimd.iota(iota_f_i[:], pattern=[[0, P], [1, W]], base=0,
                       channel_multiplier=0)
        iota_f = consts.tile([128, P * W], f32)
        nc.vector.tensor_copy(iota_f[:], iota_f_i[:])

        # --- small inputs
        # hbt broadcast across all 128 partitions (DMA-side broadcast)
        hbt_bc = consts.tile([128, 2, P * r_pad], f32)
        nc.gpsimd.dma_start(
            out=hbt_bc[:],
            in_=bass.AP(tensor=hbt.tensor, offset=0,
                        ap=[[0, 128], [P * r_pad, 2], [1, P * r_pad]]))
        # [r_pad, 14] rows -> [128 part, win, 14]
        wb_s = consts.tile([128, nwin, 2 * P], f32)
        nc.sync.dma_start(
            out=wb_s[:],
            in_=bass.AP(tensor=wb.tensor, offset=0,
                        ap=[[2 * P, 128], [RW * 2 * P, nwin], [1, 2 * P]]))
        hb_s = consts.tile([128, nwin, 2 * P], f32)
        nc.sync.dma_start(
            out=hb_s[:],
            in_=bass.AP(tensor=hb.tensor, offset=0,
                        ap=[[2 * P, 128], [RW * 2 * P, nwin], [1, 2 * P]]))

        # --- features: [h, (c, w)] SBUF-resident slab per batch (4 x 3.2 MB),
        # separate tiles so each window only waits for its own batch's DMA
        for _rep in range(repeat):
          fs = []
          for b in range(N_IMG):
            fsb = fpool.tile([128, C_SLAB, W], f32, tag=f"fs{b}")
            src = bass.AP(tensor=fslab.tensor,
                          offset=b * C_SLAB * H * W,
                          ap=[[W, H], [H * W, C_SLAB], [1, W]])
            nc.sync.dma_start(out=fsb[:], in_=src)
            fs.append(fsb)

          for win in range(nwin):
             b_win = win_batch[win]

             # Mw masks [r, (pw, w)] = (w >= ws[r,pw]) & (w < we[r,pw])
             mww = mpool.tile([128, P, W], f32, tag="mww")
             t0 = mpool.tile([128, P, W], f32, tag="mt0")
             ws_ap = bass.AP(tensor=wb_s.tensor,
                             offset=wb_s.offset + win * 2 * P,
                             ap=[wb_s.ap[0], [1, P], [0, W]])
             we_ap = bass.AP(tensor=wb_s.tensor,
                             offset=wb_s.offset + win * 2 * P + P,
                             ap=[wb_s.ap[0], [1, P], [0, W]])
             nc.any.tensor_tensor(out=t0[:], in0=iota_f[:], in1=ws_ap,
                                     op=Alu.is_ge)
             nc.any.tensor_tensor(out=mww[:], in0=iota_f[:], in1=we_ap,
                                     op=Alu.is_lt)
             nc.any.tensor_tensor(out=mww[:], in0=t0[:], in1=mww[:], op=Alu.mult)

             # H-range masks [h, (ph, r)] for this window (on Pool)
             hmask = mpool.tile([128, P, RW], f32, tag="hmask")
             h_t0 = mpool.tile([128, P, RW], f32, tag="ht0")
             io_p = bass.AP(tensor=iota_p.tensor, offset=iota_p.offset,
                            ap=[iota_p.ap[0], [0, P], [0, RW]])
             hs_ap = bass.AP(tensor=hbt_bc.tensor,
                             offset=hbt_bc.offset + win * RW,
                             ap=[hbt_bc.ap[0], [r_pad, P], [1, RW]])
             he_ap = bass.AP(tensor=hbt_bc.tensor,
                             offset=hbt_bc.offset + P * r_pad + win * RW,
                             ap=[hbt_bc.ap[0], [r_pad, P], [1, RW]])
             nc.any.tensor_tensor(out=h_t0[:], in0=io_p, in1=hs_ap,
                                     op=Alu.is_ge)
             nc.any.tensor_tensor(out=hmask[:], in0=io_p, in1=he_ap,
                                     op=Alu.is_lt)
             nc.any.tensor_tensor(out=hmask[:], in0=h_t0[:], in1=hmask[:],
                                     op=Alu.mult)

             bins = bpool.tile([128, C_SLAB], f32, tag="bins")

             for ph in range(P):
                 u = psum.tile([128, P, W], f32, tag="u")
                 for (n0, n1) in ((0, 4), (4, P)):
                     nc.tensor.matmul(
                         out=u[:, n0:n1, :],
                         lhsT=hmask[:, ph, :],
                         rhs=fs[b_win][:, ph * P + n0:ph * P + n1, :],
                         start=True, stop=True)
                 v = spool.tile([128, P, W], f32, tag="v")
                 nc.any.tensor_tensor(out=v[:], in0=u[:], in1=mww[:],
                                         op=Alu.mult)
                 nc.vector.tensor_reduce(
                     out=bins[:, ph * P:(ph + 1) * P],
                     in_=v[:], axis=mybir.AxisListType.X, op=Alu.add)

             # area = (he-hs)*(we-ws); out = bins / max(area,1) * (area>0)
             hd = spool.tile([128, P], f32, tag="hd")
             wd = spool.tile([128, P], f32, tag="wd")
             nc.any.tensor_tensor(
                 out=hd[:], in0=hb_s[:, win, P:2 * P], in1=hb_s[:, win, 0:P],
                 op=Alu.subtract)
             nc.any.tensor_tensor(
                 out=wd[:], in0=wb_s[:, win, P:2 * P], in1=wb_s[:, win, 0:P],
                 op=Alu.subtract)
             area = spool.tile([128, C_SLAB], f32, tag="area")
             hd_ap = bass.AP(tensor=hd.tensor, offset=hd.offset,
                             ap=[hd.ap[0], [1, P], [0, P]])
             wd_ap = bass.AP(tensor=wd.tensor, offset=wd.offset,
                             ap=[wd.ap[0], [0, P], [1, P]])
             nc.any.tensor_tensor(out=area[:], in0=hd_ap, in1=wd_ap, op=Alu.mult)
             denom = spool.tile([128, C_SLAB], f32, tag="denom")
             nc.any.tensor_scalar_max(denom[:], area[:], 1.0)
             rec = spool.tile([128, C_SLAB], f32, tag="rec")
             nc.vector.reciprocal(rec[:], denom[:])
             posm = spool.tile([128, C_SLAB], f32, tag="posm")
             nc.any.tensor_scalar(out=posm[:], in0=area[:], scalar1=0.0,
                                     scalar2=None, op0=Alu.is_gt)
             outw = opool.tile([128, C_SLAB], f32, tag="outw")
             nc.any.tensor_tensor(out=outw[:], in0=bins[:], in1=rec[:],
                                     op=Alu.mult)
             nc.any.tensor_tensor(out=outw[:], in0=outw[:], in1=posm[:],
                                     op=Alu.mult)
             nc.sync.dma_start(out=out[win * RW:(win + 1) * RW, :], in_=outw[:])

    nc.compile()
    return nc


_PROG_CACHE = {}


def _get_program(win_batch):
    key = tuple(win_batch)
    if key not in _PROG_CACHE:
        _PROG_CACHE[key] = build_program(win_batch)
    return _PROG_CACHE[key]


def _prep(rois: np.ndarray):
    batch = rois[:, 0].astype(np.int32)
    padmap, win_batch = _plan(batch)
    rois_p = rois[padmap]
    hs, he, ws, we = _bounds(rois_p)
    hbt = np.stack([hs.T.reshape(-1), he.T.reshape(-1)])  # [2, 7*r_pad]
    wb = np.concatenate([ws, we], axis=1)                 # [r_pad, 14]
    hb = np.concatenate([hs, he], axis=1)                 # [r_pad, 14]
    return padmap, win_batch, hbt, wb, hb


# ---------------------------------------------------------------- entrypoint
def kernel(features: np.ndarray, rois: np.ndarray) -> np.ndarray:
    from concourse.bass_utils import run_bass_kernel_spmd

    features = np.asarray(features, dtype=np.float32)
    rois = np.asarray(rois, dtype=np.float32)

    padmap, win_batch, hbt, wb, hb = _prep(rois)
    nc = _get_program(win_batch)

    in_maps = []
    for k in range(NCORES):
        in_maps.append({
            "fslab": np.ascontiguousarray(
                features[:, k * C_SLAB:(k + 1) * C_SLAB]),
            "hbt": np.ascontiguousarray(hbt),
            "wb": np.ascontiguousarray(wb),
            "hb": np.ascontiguousarray(hb),
        })

    res = run_bass_kernel_spmd(nc, in_maps, list(range(NCORES))).results

    result = np.empty((R, C_OUT, C_SLAB), dtype=np.float32)
    for k in range(NCORES):
        # scatter padded rows back; padded duplicates overwrite harmlessly
        result[padmap, k, :] = res[k]["out"]
    return result.reshape(R, C_OUT, P, P)



# revision 3
# speedup vs baseline: 5.8914x; 5.8914x over previous
"""PSRoIPool (position-sensitive ROI pooling) for Trainium2, 8 NeuronCores.

Problem (hardcoded):
  features [4, 392, 128, 128] f32, rois [512, 5] f32 (batch, x1, y1, x2, y2)
  out [512, 8, 7, 7] f32;  C = C_out(8) * 7 * 7;  spatial_scale = 1/16.

Sharding: by output channel c_out (8 cores). Core k owns feature channels
[49k, 49k+49) of every image (1/8 of the features, read exactly once as
bf16) and computes out[:, k, :, :] for ALL 512 rois.

Algorithm (per core):
  - rois stable-sorted by batch -> exactly 4 windows of 128 rois. A window
    spanning a batch boundary is computed as 2 PSUM-accumulated matmul
    passes whose H-masks are zeroed outside their roi range (host-side).
  - stage 1 (PE, bf16): u[r, pw, w] = sum_h Mh[h, r; ph] * F[h, c_s, w]
    per (window, ph); H-mask is the stationary operand.
  - stage 2 (DVE, one fused pass): custom DVE op MASKED_PREFIX_ANT computes
    P[r, t] = prefix-sum of u[r, t] * Mw[r, t] over the flat (pw, w) axis.
    Bin sums drop out as strided differences at the 128-column boundaries:
      bins[r, ph, 0]    = P[r, 127]
      bins[r, ph, pw>0] = P[r, 128(pw+1)-1] - P[r, 128 pw - 1]
  - out = bins * recip_area (recip precomputed on host, 0 for empty bins).

All masks + reciprocal areas are precomputed on the host and DMAed (they
depend only on the rois, are identical on all 8 cores, and cost ~2.7 MB
vs 6.4 MB of bf16 features).
"""

import numpy as np
from contextlib import ExitStack

try:
    import ml_dtypes
    _BF16 = ml_dtypes.bfloat16
except ImportError:  # pragma: no cover
    import jax.numpy as _jnp
    _BF16 = _jnp.bfloat16

N_IMG, C_FULL, H, W = 4, 392, 128, 128
R = 512
P = 7  # OUT_SIZE == GROUP
C_OUT = 8
C_SLAB = P * P  # 49 channels per core
SCALE = np.float32(0.0625)
NCORES = 8
RW = 128  # rois per window
NWIN = R // RW

OP_NAME = "MASKED_PREFIX_ANT"


# ------------------------------------------------------------- custom DVE op
def _get_custom_op():
    """out[p, k] = sum_{j<=k} in0[p, j] * in1[p, j] (fp32 internal state).

    Registered by appending to concourse.dve_ops.OPS (the per-NEFF DVE
    table is generated from OPS by bass_utils.dve_table_for_ops)."""
    import concourse.dve_ops as D

    for op in D.OPS:
        if op.name == OP_NAME:
            return op

    from concourse.dve_spec import Spec, scan, AluOp, Src0, Src1, lower, _has_src1
    from concourse.dve_uop import DveOpSpec

    def ref(in0, in1, s0, s1, imm2):
        p = in0.astype(np.float32) * in1.astype(np.float32)
        return np.add.accumulate(p, axis=-1).astype(np.float32)

    spec = Spec(body=scan(AluOp.ADD, Src0 * Src1), reference=ref)
    row = max(D._SUB_OPCODE_FOR_NAME.values()) + 1
    assert row < 0x20
    shas = {}
    for ver in ("v3", "v4"):
        s = DveOpSpec(name=OP_NAME, opcode=row,
                      uops=lower(spec, ver=ver), rd1_en=_has_src1(spec))
        shas[ver] = s.sha(ver)
    op = D.DveOp(OP_NAME, spec, subdim=False, uops_sha=shas)
    D.OPS.append(op)
    D.CUSTOM_DVE_SPECS[op.name] = op.spec
    D._SUB_OPCODE_FOR_NAME[op.name] = row
    return op


# ---------------------------------------------------------------- host math
def _bounds(rois_sorted: np.ndarray):
    """Bit-exact f32 mirror of the reference coordinate transform."""
    r = rois_sorted.astype(np.float32)
    one = np.float32(1.0)
    rsw = np.round(r[:, 1]) * SCALE
    rsh = np.round(r[:, 2]) * SCALE
    rew = (np.round(r[:, 3]) + one) * SCALE
    reh = (np.round(r[:, 4]) + one) * SCALE
    roi_w = np.maximum(rew - rsw, np.float32(0.1))
    roi_h = np.maximum(reh - rsh, np.float32(0.1))
    bin_w = (roi_w / np.float32(P)).astype(np.float32)
    bin_h = (roi_h / np.float32(P)).astype(np.float32)
    p = np.arange(P, dtype=np.float32)
    hs = np.clip(np.floor(p[None, :] * bin_h[:, None] + rsh[:, None]), 0, H)
    he = np.clip(np.ceil((p[None, :] + one) * bin_h[:, None] + rsh[:, None]), 0, H)
    ws = np.clip(np.floor(p[None, :] * bin_w[:, None] + rsw[:, None]), 0, W)
    we = np.clip(np.ceil((p[None, :] + one) * bin_w[:, None] + rsw[:, None]), 0, W)
    return hs, he, ws, we  # [R, P] f32 (integer-valued)


def _plan(batch: np.ndarray):
    """Pack rois into NWIN windows of RW, minimising matmul streams: pull
    whole-window pure-batch groups first, then chunk the remainders.
    Returns (order [R], pieces: per-window ((b, c0, c1), ...))."""
    groups = [np.nonzero(batch == b)[0] for b in range(N_IMG)]
    pure, rem = [], []
    for idx in groups:
        n_pure = len(idx) // RW
        for i in range(n_pure):
            pure.append(idx[i * RW:(i + 1) * RW])
        rem.append(idx[n_pure * RW:])
    rem = np.concatenate([r for r in rem if len(r)]) if any(
        len(r) for r in rem) else np.empty(0, np.int64)
    order = np.concatenate(pure + ([rem] if len(rem) else []))
    assert len(order) == R
    sb = batch[order]
    pieces = []
    for w in range(NWIN):
        seg = sb[w * RW:(w + 1) * RW]
        ps = []
        start = 0
        for i in range(1, RW + 1):
            if i == RW or seg[i] != seg[start]:
                ps.append((int(seg[start]), start, i))
                start = i
        pieces.append(tuple(ps))
    return order, tuple(pieces)


def _prep(rois: np.ndarray):
    batch = rois[:, 0].astype(np.int32)
    order, pieces = _plan(batch)
    rs = rois[order]
    hs, he, ws, we = _bounds(rs)

    harange = np.arange(H, dtype=np.float32)
    warange = np.arange(W, dtype=np.float32)

    # hmask streams: one per (window, piece): [h, ph, r] zeroed outside piece
    hm_list = []
    for w, ps in enumerate(pieces):
        sl = slice(w * RW, (w + 1) * RW)
        hsw, hew = hs[sl], he[sl]  # [RW, P]
        m = ((harange[:, None, None] >= hsw.T[None, :, :])
             & (harange[:, None, None] < hew.T[None, :, :]))  # [H, P, RW]
        for (b, c0, c1) in ps:
            mm = np.zeros((H, P, RW), dtype=np.float32)
            mm[:, :, c0:c1] = m[:, :, c0:c1]
            hm_list.append(mm.reshape(H, P * RW))
    hmask = np.asarray(np.stack(hm_list), dtype=_BF16)  # [NS, 128, 896]

    # W-interval mask per window: [r, pw, w]
    mw = ((warange[None, None, :] >= ws[:, :, None])
          & (warange[None, None, :] < we[:, :, None])).astype(np.float32)
    mww = np.asarray(mw.reshape(NWIN, RW, P * W), dtype=_BF16)

    # reciprocal area, 0 where empty  [NWIN, RW, 49] (cs = ph*7+pw)
    ah = he - hs  # [R, P]
    aw = we - ws
    area = ah[:, :, None] * aw[:, None, :]  # [R, ph, pw]
    recip = np.where(area > 0, np.float32(1.0) / np.maximum(area, 1.0),
                     np.float32(0.0)).astype(np.float32)
    recip = recip.reshape(NWIN, RW, C_SLAB)

    return order, pieces, hmask, mww, recip


# ---------------------------------------------------------------- device IR
def build_program(pieces, repeat=1):
    import concourse.bass as bass
    import concourse.tile as tile
    from concourse import bacc, mybir

    f32 = mybir.dt.float32
    bf16 = mybir.dt.bfloat16
    Alu = mybir.AluOpType
    op = _get_custom_op()

    nstream = sum(len(ps) for ps in pieces)
    CW = C_SLAB * W  # 6272

    nc = bacc.Bacc("TRN2", target_bir_lowering=False, debug=False,
                   num_devices=NCORES)

    # [N, H, CS, W] so each partition line is CS*W contiguous bytes
    fslab = nc.dram_tensor("fslab", [N_IMG, H, C_SLAB, W], bf16,
                           kind="ExternalInput").ap()
    hmask = nc.dram_tensor("hmask", [nstream, H, P * RW], bf16,
                           kind="ExternalInput").ap()
    mww = nc.dram_tensor("mww", [NWIN, RW, P * W], bf16,
                         kind="ExternalInput").ap()
    recip = nc.dram_tensor("recip", [NWIN, RW, C_SLAB], f32,
                           kind="ExternalInput").ap()
    out = nc.dram_tensor("out", [R, C_SLAB], f32, kind="ExternalOutput").ap()

    with tile.TileContext(nc) as tc, ExitStack() as ctx:
        fpool = ctx.enter_context(tc.tile_pool(name="fs", bufs=2))
        mpool = ctx.enter_context(tc.tile_pool(name="masks", bufs=2))
        spool = ctx.enter_context(tc.tile_pool(name="scratch", bufs=3))
        bpool = ctx.enter_context(tc.tile_pool(name="bins", bufs=2))
        opool = ctx.enter_context(tc.tile_pool(name="outw", bufs=2))
        psum = ctx.enter_context(tc.tile_pool(name="ps", bufs=3, space="PSUM"))

        for _rep in range(repeat):
            fs = []
            for b in range(N_IMG):
                fsb = fpool.tile([128, C_SLAB, W], bf16, tag=f"fs{b}")
                src = bass.AP(tensor=fslab.tensor, offset=b * H * CW,
                              ap=[[CW, H], [W, C_SLAB], [1, W]])
                nc.sync.dma_start(out=fsb[:], in_=src)
                fs.append(fsb)

            sidx = 0
            for win in range(NWIN):
                ps = pieces[win]
                hm = []
                for _ in ps:
                    hmt = mpool.tile([128, P * RW], bf16, tag=f"hm{sidx}")
                    nc.sync.dma_start(
                        out=hmt[:],
                        in_=bass.AP(tensor=hmask.tensor,
                                    offset=sidx * H * P * RW,
                                    ap=[[P * RW, H], [1, P * RW]]))
                    hm.append(hmt)
                    sidx += 1
                mwt = mpool.tile([128, P * W], bf16, tag="mw")
                nc.sync.dma_start(
                    out=mwt[:],
                    in_=bass.AP(tensor=mww.tensor, offset=win * RW * P * W,
                                ap=[[P * W, RW], [1, P * W]]))
                rct = mpool.tile([128, C_SLAB], f32, tag="rc")
                nc.sync.dma_start(
                    out=rct[:],
                    in_=bass.AP(tensor=recip.tensor, offset=win * RW * C_SLAB,
                                ap=[[C_SLAB, RW], [1, C_SLAB]]))

                bins = bpool.tile([128, C_SLAB], f32, tag="bins")

                for ph in range(P):
                    u = psum.tile([128, P, W], f32, tag="u")
                    for (n0, n1) in ((0, 4), (4, P)):
                        for i, (b, c0, c1) in enumerate(ps):
                            nc.tensor.matmul(
                                out=u[:, n0:n1, :],
                                lhsT=hm[i][:, ph * RW:(ph + 1) * RW],
                                rhs=fs[b][:, ph * P + n0:ph * P + n1, :],
                                start=(i == 0), stop=(i == len(ps) - 1))
                    pre = spool.tile([128, P * W], f32, tag="pre")
                    nc.vector._custom_dve(op, out=pre[:], in0=u[:], in1=mwt[:])
                    cs0 = ph * P
                    nc.scalar.copy(bins[:, cs0:cs0 + 1], pre[:, W - 1:W])
                    hi = bass.AP(tensor=pre.tensor, offset=pre.offset + 2 * W - 1,
                                 ap=[pre.ap[0], [W, P - 1]])
                    lo = bass.AP(tensor=pre.tensor, offset=pre.offset + W - 1,
                                 ap=[pre.ap[0], [W, P - 1]])
                    nc.gpsimd.tensor_tensor(out=bins[:, cs0 + 1:cs0 + P],
                                            in0=hi, in1=lo, op=Alu.subtract)

                outw = opool.tile([128, C_SLAB], f32, tag="outw")
                nc.gpsimd.tensor_tensor(out=outw[:], in0=bins[:], in1=rct[:],
                                        op=Alu.mult)
                nc.sync.dma_start(out=out[win * RW:(win + 1) * RW, :],
                                  in_=outw[:])

    nc.compile()
    return nc


_PROG_CACHE = {}


def _get_program(pieces, repeat=1):
    key = (pieces, repeat)
    if key not in _PROG_CACHE:
        _PROG_CACHE[key] = build_program(pieces, repeat=repeat)
    return _PROG_CACHE[key]


# ---------------------------------------------------------------- entrypoint
def kernel(features: np.ndarray, rois: np.ndarray) -> np.ndarray:
    from concourse.bass_utils import run_bass_kernel_spmd

    features = np.asarray(features, dtype=np.float32)
    rois = np.asarray(rois, dtype=np.float32)

    order, pieces, hmask, mww, recip = _prep(rois)
    nc = _get_program(pieces)

    # [N, CS, H, W] -> [N, H, CS, W], bf16
    fbf = features.astype(_BF16)
    in_maps = []
    for k in range(NCORES):
        slab = fbf[:, k * C_SLAB:(k + 1) * C_SLAB].transpose(0, 2, 1, 3)
        in_maps.append({
            "fslab": np.ascontiguousarray(slab),
            "hmask": hmask,
            "mww": mww,
            "recip": recip,
        })

    res = run_bass_kernel_spmd(nc, in_maps, list(range(NCORES))).results

    result = np.empty((R, C_OUT, C_SLAB), dtype=np.float32)
    for k in range(NCORES):
        result[order, k, :] = res[k]["out"]
    return result.reshape(R, C_OUT, P, P)


# revision 6
# speedup vs baseline: 11.0132x; 1.8694x over previous
"""PSRoIPool (position-sensitive ROI pooling) for Trainium2, 8 NeuronCores.

Problem (hardcoded):
  features [4, 392, 128, 128] f32, rois [512, 5] f32 (batch, x1, y1, x2, y2)
  out [512, 8, 7, 7] f32;  C = C_out(8) * 7 * 7;  spatial_scale = 1/16.

Sharding: by output channel c_out (8 cores). Core k owns feature channels
[49k, 49k+49) of every image (1/8 of the features, read exactly once as
bf16) and computes out[:, k, :, :] for ALL 512 rois.

Algorithm (per core):
  - rois stable-sorted by batch -> exactly 4 windows of 128 rois. A window
    spanning a batch boundary is computed as 2 PSUM-accumulated matmul
    passes whose H-masks are zeroed outside their roi range (host-side).
  - stage 1 (PE, bf16): u[r, pw, w] = sum_h Mh[h, r; ph] * F[h, c_s, w]
    per (window, ph); H-mask is the stationary operand.
  - stage 2 (DVE, one fused pass): custom DVE op MASKED_PREFIX_ANT computes
    P[r, t] = prefix-sum of u[r, t] * Mw[r, t] over the flat (pw, w) axis.
    Bin sums drop out as strided differences at the 128-column boundaries:
      bins[r, ph, 0]    = P[r, 127]
      bins[r, ph, pw>0] = P[r, 128(pw+1)-1] - P[r, 128 pw - 1]
  - out = bins * recip_area (recip precomputed on host, 0 for empty bins).

All masks + reciprocal areas are precomputed on the host and DMAed (they
depend only on the rois, are identical on all 8 cores, and cost ~2.7 MB
vs 6.4 MB of bf16 features).
"""

import numpy as np
from contextlib import ExitStack

try:
    import ml_dtypes
    _BF16 = ml_dtypes.bfloat16
except ImportError:  # pragma: no cover
    import jax.numpy as _jnp
    _BF16 = _jnp.bfloat16

N_IMG, C_FULL, H, W = 4, 392, 128, 128
R = 512
P = 7  # OUT_SIZE == GROUP
C_OUT = 8
C_SLAB = P * P  # 49 channels per core
SCALE = np.float32(0.0625)
NCORES = 8
RW = 128  # rois per window
NWIN = R // RW

OP_NAME = "MASKED_PREFIX_ANT"


# ------------------------------------------------------------- custom DVE op
def _get_custom_op():
    """out[p, k] = sum_{j<=k} in0[p, j] * in1[p, j] (fp32 internal state).

    Registered by appending to concourse.dve_ops.OPS (the per-NEFF DVE
    table is generated from OPS by bass_utils.dve_table_for_ops)."""
    import concourse.dve_ops as D

    for op in D.OPS:
        if op.name == OP_NAME:
            return op

    from concourse.dve_spec import Spec, scan, AluOp, Src0, Src1, lower, _has_src1
    from concourse.dve_uop import DveOpSpec

    def ref(in0, in1, s0, s1, imm2):
        p = in0.astype(np.float32) * in1.astype(np.float32)
        return np.add.accumulate(p, axis=-1).astype(np.float32)

    spec = Spec(body=scan(AluOp.ADD, Src0 * Src1), reference=ref)
    row = max(D._SUB_OPCODE_FOR_NAME.values()) + 1
    assert row < 0x20
    shas = {}
    for ver in ("v3", "v4"):
        s = DveOpSpec(name=OP_NAME, opcode=row,
                      uops=lower(spec, ver=ver), rd1_en=_has_src1(spec))
        shas[ver] = s.sha(ver)
    op = D.DveOp(OP_NAME, spec, subdim=False, uops_sha=shas)
    D.OPS.append(op)
    D.CUSTOM_DVE_SPECS[op.name] = op.spec
    D._SUB_OPCODE_FOR_NAME[op.name] = row
    return op


# ---------------------------------------------------------------- host math
def _bounds(rois_sorted: np.ndarray):
    """Bit-exact f32 mirror of the reference coordinate transform."""
    r = rois_sorted.astype(np.float32)
    one = np.float32(1.0)
    rsw = np.round(r[:, 1]) * SCALE
    rsh = np.round(r[:, 2]) * SCALE
    rew = (np.round(r[:, 3]) + one) * SCALE
    reh = (np.round(r[:, 4]) + one) * SCALE
    roi_w = np.maximum(rew - rsw, np.float32(0.1))
    roi_h = np.maximum(reh - rsh, np.float32(0.1))
    bin_w = (roi_w / np.float32(P)).astype(np.float32)
    bin_h = (roi_h / np.float32(P)).astype(np.float32)
    p = np.arange(P, dtype=np.float32)
    hs = np.clip(np.floor(p[None, :] * bin_h[:, None] + rsh[:, None]), 0, H)
    he = np.clip(np.ceil((p[None, :] + one) * bin_h[:, None] + rsh[:, None]), 0, H)
    ws = np.clip(np.floor(p[None, :] * bin_w[:, None] + rsw[:, None]), 0, W)
    we = np.clip(np.ceil((p[None, :] + one) * bin_w[:, None] + rsw[:, None]), 0, W)
    return hs, he, ws, we  # [R, P] f32 (integer-valued)


def _plan(batch: np.ndarray):
    """Pack rois into NWIN windows of RW, minimising matmul streams: pull
    whole-window pure-batch groups first, then chunk the remainders.
    Returns (order [R], pieces: per-window ((b, c0, c1), ...))."""
    groups = [np.nonzero(batch == b)[0] for b in range(N_IMG)]
    pure, rem = [], []
    for idx in groups:
        n_pure = len(idx) // RW
        for i in range(n_pure):
            pure.append(idx[i * RW:(i + 1) * RW])
        rem.append(idx[n_pure * RW:])
    rem = np.concatenate([r for r in rem if len(r)]) if any(
        len(r) for r in rem) else np.empty(0, np.int64)
    order = np.concatenate(pure + ([rem] if len(rem) else []))
    assert len(order) == R
    sb = batch[order]
    pieces = []
    for w in range(NWIN):
        seg = sb[w * RW:(w + 1) * RW]
        ps = []
        start = 0
        for i in range(1, RW + 1):
            if i == RW or seg[i] != seg[start]:
                ps.append((int(seg[start]), start, i))
                start = i
        pieces.append(tuple(ps))
    return order, tuple(pieces)


def _prep(rois: np.ndarray):
    batch = rois[:, 0].astype(np.int32)
    order, pieces = _plan(batch)
    rs = rois[order]
    hs, he, ws, we = _bounds(rs)

    harange = np.arange(H, dtype=np.float32)
    warange = np.arange(W, dtype=np.float32)

    # hmask streams: one per (window, piece): [h, ph, r] zeroed outside piece
    hm_list = []
    for w, ps in enumerate(pieces):
        sl = slice(w * RW, (w + 1) * RW)
        hsw, hew = hs[sl], he[sl]  # [RW, P]
        m = ((harange[:, None, None] >= hsw.T[None, :, :])
             & (harange[:, None, None] < hew.T[None, :, :]))  # [H, P, RW]
        for (b, c0, c1) in ps:
            mm = np.zeros((H, P, RW), dtype=np.float32)
            mm[:, :, c0:c1] = m[:, :, c0:c1]
            hm_list.append(mm.reshape(H, P * RW))
    hmask = np.asarray(np.stack(hm_list), dtype=_BF16)  # [NS, 128, 896]

    # W-interval mask per window: [r, pw, w]
    mw = ((warange[None, None, :] >= ws[:, :, None])
          & (warange[None, None, :] < we[:, :, None])).astype(np.float32)
    mww = np.asarray(mw.reshape(NWIN, RW, P * W), dtype=_BF16)

    # reciprocal area, 0 where empty  [NWIN, RW, 49] (cs = ph*7+pw)
    ah = he - hs  # [R, P]
    aw = we - ws
    area = ah[:, :, None] * aw[:, None, :]  # [R, ph, pw]
    recip = np.where(area > 0, np.float32(1.0) / np.maximum(area, 1.0),
                     np.float32(0.0)).astype(np.float32)
    recip = recip.reshape(NWIN, RW, C_SLAB)

    return order, pieces, hmask, mww, recip


# ---------------------------------------------------------------- device IR
def build_program(pieces, repeat=1, skip_dve=False, dma_once=False):
    import concourse.bass as bass
    import concourse.tile as tile
    from concourse import bacc, mybir

    f32 = mybir.dt.float32
    bf16 = mybir.dt.bfloat16
    Alu = mybir.AluOpType
    op = _get_custom_op()

    nstream = sum(len(ps) for ps in pieces)
    CW = C_SLAB * W  # 6272

    nc = bacc.Bacc("TRN2", target_bir_lowering=False, debug=False,
                   num_devices=NCORES)

    # [N, H, CS, W] so each partition line is CS*W contiguous bytes
    fslab = nc.dram_tensor("fslab", [N_IMG, H, C_SLAB, W], bf16,
                           kind="ExternalInput").ap()
    hmask = nc.dram_tensor("hmask", [nstream, H, P * RW], bf16,
                           kind="ExternalInput").ap()
    mww = nc.dram_tensor("mww", [NWIN, RW, P * W], bf16,
                         kind="ExternalInput").ap()
    recip = nc.dram_tensor("recip", [NWIN, RW, C_SLAB], f32,
                           kind="ExternalInput").ap()
    out = nc.dram_tensor("out", [R, C_SLAB], f32, kind="ExternalOutput").ap()

    with tile.TileContext(nc) as tc, ExitStack() as ctx:
        fpool = ctx.enter_context(tc.tile_pool(name="fs", bufs=2))
        mpool = ctx.enter_context(tc.tile_pool(name="masks", bufs=2))
        spool = ctx.enter_context(tc.tile_pool(name="scratch", bufs=3))
        bpool = ctx.enter_context(tc.tile_pool(name="bins", bufs=2))
        opool = ctx.enter_context(tc.tile_pool(name="outw", bufs=2))
        psum = ctx.enter_context(tc.tile_pool(name="ps", bufs=3, space="PSUM"))

        for _rep in range(repeat):
            if _rep == 0 or not dma_once:
                fs = []
                for b in range(N_IMG):
                    fsb = fpool.tile([128, C_SLAB, W], bf16, tag=f"fs{b}")
                    src = bass.AP(tensor=fslab.tensor, offset=b * H * CW,
                                  ap=[[CW, H], [W, C_SLAB], [1, W]])
                    nc.sync.dma_start(out=fsb[:], in_=src)
                    fs.append(fsb)

            sidx = 0
            for win in range(NWIN):
                ps = pieces[win]
                hm = []
                for _ in ps:
                    hmt = mpool.tile([128, P * RW], bf16, tag=f"hm{sidx}")
                    nc.sync.dma_start(
                        out=hmt[:],
                        in_=bass.AP(tensor=hmask.tensor,
                                    offset=sidx * H * P * RW,
                                    ap=[[P * RW, H], [1, P * RW]]))
                    hm.append(hmt)
                    sidx += 1
                mwt = mpool.tile([128, P * W], bf16, tag="mw")
                nc.sync.dma_start(
                    out=mwt[:],
                    in_=bass.AP(tensor=mww.tensor, offset=win * RW * P * W,
                                ap=[[P * W, RW], [1, P * W]]))
                rct = mpool.tile([128, C_SLAB], f32, tag="rc")
                nc.sync.dma_start(
                    out=rct[:],
                    in_=bass.AP(tensor=recip.tensor, offset=win * RW * C_SLAB,
                                ap=[[C_SLAB, RW], [1, C_SLAB]]))

                bins = bpool.tile([128, C_SLAB], f32, tag="bins")

                for ph in range(P):
                    u = psum.tile([128, P, W], f32, tag="u")
                    for (n0, n1) in ((0, 4), (4, P)):
                        for i, (b, c0, c1) in enumerate(ps):
                            nc.tensor.matmul(
                                out=u[:, n0:n1, :],
                                lhsT=hm[i][:, ph * RW:(ph + 1) * RW],
                                rhs=fs[b][:, ph * P + n0:ph * P + n1, :],
                                start=(i == 0), stop=(i == len(ps) - 1))
                    pre = spool.tile([128, P * W], f32, tag="pre")
                    if skip_dve:
                        # experiment: bypass stage-2 (wrong results, timing only)
                        nc.gpsimd.tensor_tensor(
                            out=pre[:, 0:P], in0=u[:, 0, 0:P], in1=u[:, 1, 0:P],
                            op=Alu.add)
                    else:
                        nc.vector._custom_dve(op, out=pre[:], in0=u[:], in1=mwt[:])
                    cs0 = ph * P
                    nc.scalar.copy(bins[:, cs0:cs0 + 1], pre[:, W - 1:W])
                    hi = bass.AP(tensor=pre.tensor, offset=pre.offset + 2 * W - 1,
                                 ap=[pre.ap[0], [W, P - 1]])
                    lo = bass.AP(tensor=pre.tensor, offset=pre.offset + W - 1,
                                 ap=[pre.ap[0], [W, P - 1]])
                    nc.gpsimd.tensor_tensor(out=bins[:, cs0 + 1:cs0 + P],
                                            in0=hi, in1=lo, op=Alu.subtract)

                outw = opool.tile([128, C_SLAB], f32, tag="outw")
                nc.gpsimd.tensor_tensor(out=outw[:], in0=bins[:], in1=rct[:],
                                        op=Alu.mult)
                nc.sync.dma_start(out=out[win * RW:(win + 1) * RW, :],
                                  in_=outw[:])

    nc.compile()
    return nc


_PROG_CACHE = {}


def _get_program(pieces, repeat=1):
    key = (pieces, repeat)
    if key not in _PROG_CACHE:
        _PROG_CACHE[key] = build_program(pieces, repeat=repeat)
    return _PROG_CACHE[key]


# ---------------------------------------------------------------- entrypoint
def kernel(features: np.ndarray, rois: np.ndarray) -> np.ndarray:
    from concourse.bass_utils import run_bass_kernel_spmd

    features = np.asarray(features, dtype=np.float32)
    rois = np.asarray(rois, dtype=np.float32)

    order, pieces, hmask, mww, recip = _prep(rois)
    nc = _get_program(pieces)

    # [N, CS, H, W] -> [N, H, CS, W], bf16
    fbf = features.astype(_BF16)
    in_maps = []
    for k in range(NCORES):
        slab = fbf[:, k * C_SLAB:(k + 1) * C_SLAB].transpose(0, 2, 1, 3)
        in_maps.append({
            "fslab": np.ascontiguousarray(slab),
            "hmask": hmask,
            "mww": mww,
            "recip": recip,
        })

    res = run_bass_kernel_spmd(nc, in_maps, list(range(NCORES))).results

    result = np.empty((R, C_OUT, C_SLAB), dtype=np.float32)
    for k in range(NCORES):
        result[order, k, :] = res[k]["out"]
    return result.reshape(R, C_OUT, P, P)
